# revision 1
# baseline (speedup 1.0000x reference)
import os
import sys
from contextlib import ExitStack

import numpy as np

for _p in ("/opt/trn_rl_repo", "/root/.axon_site/_ro/trn_rl_repo"):
    if os.path.isdir(_p) and _p not in sys.path:
        sys.path.insert(0, _p)

import concourse.bass as bass
import concourse.bacc as bacc
from concourse import mybir
from concourse.tile import TileContext
from concourse.tile_rust import add_dep_helper
from concourse.bass_utils import run_bass_kernel_spmd

EPS = 1e-6
N_CORES = 8
NI = NJ = 5000
KDIM = 32
MI = MJ = 2500
NE = 200000

JPAD = 2560            # padded j extent (20 * 128)
JT = JPAD // 128       # 20 j-chunks
IPC = MI // N_CORES if MI % N_CORES == 0 else (MI + N_CORES - 1) // N_CORES
IPC = 320              # per-core i rows (2560 / 8)
ITILES = 3             # ceil(320/128)
IT_PAD = ITILES * 128  # 384
EPC = NE // N_CORES    # 25000 edges per core
QB = 196               # edge q-blocks: 196*128 = 25088 >= 25000
EPADC = QB * 128
NT_PAD = 5056          # table rows (>= 5001)

F32 = mybir.dt.float32
I32 = mybir.dt.int32
BF16 = mybir.dt.bfloat16
F32R = mybir.dt.float32r

_NC_CACHE = {}
LAST_RESULT = None


def _build_bass():
    if "nc" in _NC_CACHE:
        return _NC_CACHE["nc"]
    nc = bacc.Bacc("TRN2")
    lhs = nc.declare_dram_parameter("lhs", [33, IT_PAD], F32R, isOutput=False)
    rhs = nc.declare_dram_parameter("rhs", [33, JPAD], F32R, isOutput=False)
    bri = nc.declare_dram_parameter("bri", [128, ITILES, 2], F32, isOutput=False)
    eg = nc.declare_dram_parameter("eg", [128, JT], F32, isOutput=False)
    eib = nc.declare_dram_parameter("eib", [128, QB, 34], BF16, isOutput=False)
    ejb = nc.declare_dram_parameter("ejb", [128, QB, 34], BF16, isOutput=False)
    out = nc.declare_dram_parameter("out", [1, 2], F32, isOutput=True)

    ctx = ExitStack()
    AF = mybir.ActivationFunctionType
    with TileContext(nc) as tc:
        with (
            tc.tile_pool(name="const", bufs=1) as const,
            tc.tile_pool(name="edges", bufs=1) as epool,
            tc.tile_pool(name="dist", bufs=1) as dpool,
            tc.tile_pool(name="e1p", bufs=2) as e1pool,
            tc.tile_pool(name="small", bufs=1) as small,
            tc.tile_pool(name="pp", bufs=2, space="PSUM") as pp,
            tc.tile_pool(name="ppred", bufs=1, space="PSUM") as ppred,
        ):
            # ---- edge rows first (biggest DMA; DVE waits on these) ----
            ei_t = epool.tile([128, QB, 34], BF16, tag="ei")
            ej_t = epool.tile([128, QB, 34], BF16, tag="ej")
            QH = QB // 2
            for h in range(2):
                nc.sync.dma_start(out=ei_t[:, h * QH:(h + 1) * QH, :],
                                  in_=eib[:, h * QH:(h + 1) * QH, :])
                nc.sync.dma_start(out=ej_t[:, h * QH:(h + 1) * QH, :],
                                  in_=ejb[:, h * QH:(h + 1) * QH, :])

            # ---- load inputs ----
            lhs_t = const.tile([33, IT_PAD], F32R)
            nc.sync.dma_start(out=lhs_t[:], in_=lhs[:])
            rhs_t = const.tile([33, JPAD], F32R)
            nc.sync.dma_start(out=rhs_t[:], in_=rhs[:])
            bri_t = const.tile([128, ITILES, 2], F32)
            nc.sync.dma_start(out=bri_t[:], in_=bri[:])
            eg_t = const.tile([128, JT], F32)
            nc.sync.dma_start(out=eg_t[:], in_=eg[:])
            ones_t = const.tile([128, 1], F32)
            nc.vector.memset(ones_t[:], 1.0)
            ones_bt = const.tile([128, 1], BF16)
            nc.vector.memset(ones_bt[:], 1.0)

            # ---- pairwise: dist tiles (sqrt phase) ----
            dist_t = dpool.tile([128, ITILES, JPAD], F32)
            sqrt_instrs = []
            exp_instrs = []
            for t in range(ITILES):
                for h in range(2):
                    ps = pp.tile([128, 1280], F32, tag="mm")
                    base = h * 1280
                    for s0, w in ((0, 512), (512, 512), (1024, 256)):
                        nc.tensor.matmul(
                            out=ps[:, s0 : s0 + w],
                            lhsT=lhs_t[:, t * 128 : (t + 1) * 128],
                            rhs=rhs_t[:, base + s0 : base + s0 + w],
                            start=True,
                            stop=True,
                        )
                    sqrt_instrs.append(nc.scalar.activation(
                        out=dist_t[:, t, base : base + 1280],
                        in_=ps[:],
                        func=AF.Sqrt,
                        bias=bri_t[:, t, 0:1],
                        scale=1.0,
                    ))

            # ---- edges: diff, square, tree-reduce (per half), ln/exp sqrt ----
            diff_t = epool.tile([128, QB, 34], BF16, tag="diff")
            sq_t = epool.tile([128, QB, 32], BF16, tag="sq")
            h2_t = epool.tile([128, QB, 16], BF16, tag="h2")
            d2_t = epool.tile([128, QB], F32, tag="d2")
            for h in range(2):
                qs = slice(h * QH, (h + 1) * QH)
                nc.vector.tensor_tensor(
                    out=diff_t[:, qs, :], in0=ei_t[:, qs, :], in1=ej_t[:, qs, :],
                    op=mybir.AluOpType.subtract,
                )
                nc.vector.tensor_tensor(
                    out=sq_t[:, qs, :], in0=diff_t[:, qs, 0:32],
                    in1=diff_t[:, qs, 0:32], op=mybir.AluOpType.mult,
                )
                nc.vector.tensor_tensor(
                    out=h2_t[:, qs, :], in0=sq_t[:, qs, 0:16],
                    in1=sq_t[:, qs, 16:32], op=mybir.AluOpType.add,
                )
                nc.vector.tensor_reduce(
                    out=d2_t[:, qs], in_=h2_t[:, qs, :], axis=mybir.AxisListType.X,
                    op=mybir.AluOpType.add,
                )
            lnd_t = epool.tile([128, QB], F32, tag="lnd")
            eln_i = nc.scalar.activation(out=lnd_t[:], in_=d2_t[:], func=AF.Ln)
            de_t = epool.tile([128, QB], F32, tag="de")
            esqrt_i = nc.scalar.activation(out=de_t[:], in_=lnd_t[:], func=AF.Exp,
                                           scale=0.5)
            bias_t = epool.tile([128, QB], F32, tag="bias")
            nc.vector.tensor_copy(out=bias_t[:], in_=diff_t[:, :, 32:33])
            contrib_t = epool.tile([128, QB], F32, tag="contrib")
            nc.vector.tensor_tensor(
                out=contrib_t[:], in0=bias_t[:], in1=de_t[:],
                op=mybir.AluOpType.subtract,
            )
            csum_t = epool.tile([128, 1], F32, tag="csum")
            nc.vector.tensor_reduce(
                out=csum_t[:], in_=contrib_t[:], axis=mybir.AxisListType.X,
                op=mybir.AluOpType.add,
            )
            sc2_ps = ppred.tile([128, 2], F32, tag="sc2")
            nc.tensor.matmul(
                out=sc2_ps[0:1, 1:2], lhsT=csum_t[:], rhs=ones_t[:],
                start=True, stop=True, skip_group_check=True,
            )

            # ---- pairwise: exp phase + j-reduce matmuls ----
            s_ps = ppred.tile([128, ITILES, JT], F32, tag="sps")
            for t in range(ITILES):
                e1 = e1pool.tile([128, JPAD], BF16, tag="e1")
                exp_instrs.append(nc.scalar.activation(
                    out=e1[:],
                    in_=dist_t[:, t, :],
                    func=AF.Exp,
                    bias=bri_t[:, t, 1:2],
                    scale=-1.0,
                ))
                for c in range(JT):
                    nc.tensor.matmul(
                        out=s_ps[:, t, c : c + 1],
                        lhsT=e1[:, c * 128 : (c + 1) * 128],
                        rhs=ones_bt[:],
                        start=True,
                        stop=True,
                        skip_group_check=True,
                    )

            prod_t = small.tile([128, ITILES, JT], F32)
            eg_bc = bass.AP(
                tensor=eg_t[:].tensor,
                offset=eg_t[:].offset,
                ap=[eg_t[:].ap[0], [0, ITILES], eg_t[:].ap[1]],
            )
            nc.vector.tensor_tensor(
                out=prod_t[:], in0=s_ps[:], in1=eg_bc,
                op=mybir.AluOpType.mult,
            )
            rsum_t = small.tile([128, 1], F32)
            nc.vector.tensor_reduce(
                out=rsum_t[:], in_=prod_t[:], axis=mybir.AxisListType.XY,
                op=mybir.AluOpType.add,
            )
            nc.tensor.matmul(
                out=sc2_ps[0:1, 0:1], lhsT=rsum_t[:], rhs=ones_t[:],
                start=True, stop=True, skip_group_check=True,
            )

            for si in sqrt_instrs:
                add_dep_helper(exp_instrs[0].ins, si.ins, sync=False,
                               reason="first exp waits on all pairwise sqrts")
                add_dep_helper(eln_i.ins, si.ins, sync=False,
                               reason="edge ln waits on all pairwise sqrts")
            out_t = small.tile([1, 2], F32)
            nc.vector.tensor_copy(out=out_t[:], in_=sc2_ps[0:1, 0:2])
            nc.sync.dma_start(out=out[:], in_=out_t[:])
    ctx.close()
    nc.finalize()
    _NC_CACHE["nc"] = nc
    return nc


def kernel(beta, gamma, A, Z_i, Z_j, Gate, sample_i_idx, sample_j_idx,
           sparse_sample_i, sparse_sample_j, trace=False):
    global LAST_RESULT
    beta = np.asarray(beta, dtype=np.float32)
    gamma = np.asarray(gamma, dtype=np.float32)
    A = np.asarray(A, dtype=np.float32)
    Z_i = np.asarray(Z_i, dtype=np.float32)
    Z_j = np.asarray(Z_j, dtype=np.float32)
    Gate = np.asarray(Gate, dtype=np.float32)
    sii = np.asarray(sample_i_idx).astype(np.int64)
    sjj = np.asarray(sample_j_idx).astype(np.int64)
    ssi = np.asarray(sparse_sample_i).astype(np.int64)
    ssj = np.asarray(sparse_sample_j).astype(np.int64)

    # ---- host: tiny factor chain (O(n*k)) ----
    def softmax0(x):
        m = x.max(axis=0, keepdims=True)
        e = np.exp(x - m)
        return e / e.sum(axis=0, keepdims=True)

    Zi = softmax0(Z_i.astype(np.float64))
    Zj = softmax0(Z_j.astype(np.float64))
    Z = np.concatenate([Zi[:, sii], Zj[:, sjj]], axis=1)
    G = 1.0 / (1.0 + np.exp(-np.concatenate([Gate[sii, :], Gate[sjj, :]], axis=0).astype(np.float64)))
    ZG = Z.T * G
    C = ZG / ZG.sum(axis=0)
    AZC = A.astype(np.float64) @ (Z @ C)
    Xi_full = (AZC @ Zi).T  # (5000, 32)
    Xj_full = (AZC @ Zj).T

    xi = Xi_full[sii]       # (2500, 32)
    xj = Xj_full[sjj]
    u = xi + EPS
    ri = (u * u).sum(axis=1)
    cj = (xj * xj).sum(axis=1)
    bs = beta[sii].astype(np.float64)
    gs = gamma[sjj].astype(np.float64)

    # global pads
    IPADG = JPAD
    u_p = np.zeros((IPADG, KDIM)); u_p[:MI] = u
    ri_p = np.zeros(IPADG); ri_p[:MI] = ri
    bs_p = np.full(IPADG, -40.0); bs_p[:MI] = bs
    xj_p = np.zeros((JPAD, KDIM)); xj_p[:MJ] = xj
    cj_p = np.zeros(JPAD); cj_p[:MJ] = cj
    eg_p = np.zeros(JPAD); eg_p[:MJ] = np.exp(gs)

    rhs_np = np.concatenate([-2.0 * xj_p.T, cj_p[None, :]], axis=0).astype(np.float32)
    eg_np = eg_p.reshape(JT, 128).T.copy().astype(np.float32)

    # edge tables (host side, gathered below per core)
    ti_np = np.zeros((NI + 1, 34), dtype=np.float32)
    ti_np[:NI, 0:32] = Xi_full + EPS
    ti_np[:NI, 32] = beta
    tj_np = np.zeros((NJ + 1, 34), dtype=np.float32)
    tj_np[:NJ, 0:32] = Xj_full
    tj_np[:NJ, 32] = -gamma
    from ml_dtypes import bfloat16 as np_bf16

    nc = _build_bass()
    in_maps = []
    for c in range(N_CORES):
        i0 = c * IPC
        uc = np.zeros((IT_PAD, KDIM)); uc[:IPC] = u_p[i0:i0 + IPC]
        ric = np.zeros(IT_PAD); ric[:IPC] = ri_p[i0:i0 + IPC]
        bsc = np.full(IT_PAD, -40.0); bsc[:IPC] = bs_p[i0:i0 + IPC]
        lhs_np = np.concatenate([uc.T, np.ones((1, IT_PAD))], axis=0).astype(np.float32)
        bri_np = np.stack([ric.reshape(ITILES, 128).T,
                           bsc.reshape(ITILES, 128).T], axis=2).astype(np.float32)
        e0 = c * EPC
        eic = np.full(EPADC, NI, dtype=np.int64)
        eic[:EPC] = ssi[e0:e0 + EPC]
        ejc = np.full(EPADC, NJ, dtype=np.int64)
        ejc[:EPC] = ssj[e0:e0 + EPC]
        eib_np = ti_np[eic].reshape(128, QB, 34).astype(np_bf16)
        ejb_np = tj_np[ejc].reshape(128, QB, 34).astype(np_bf16)
        in_maps.append({
            "lhs": lhs_np,
            "rhs": rhs_np,
            "bri": bri_np,
            "eg": eg_np,
            "eib": eib_np,
            "ejb": ejb_np,
        })

    res = run_bass_kernel_spmd(nc, in_maps, core_ids=list(range(N_CORES)),
                               trace=trace)
    LAST_RESULT = res
    pair_total = 0.0
    links_total = 0.0
    for r in res.results:
        o = np.asarray(r["out"], dtype=np.float64).reshape(2)
        pair_total += o[0]
        links_total += o[1]
    return np.float32(links_total - pair_total)

